# revision 23
# baseline (speedup 1.0000x reference)
"""GPT forward kernel for 8 TRN2 NeuronCores.

Data-parallel over batch (B=8 -> 1 sequence per core). Host pre-transposes
weights (contraction dim on SBUF partitions), casts to bf16, folds the
layernorm gains into the consuming weight matrices, and performs the (tiny)
embedding gather. The device kernel keeps the residual stream transposed
(x^T [D, T], fp32 in SBUF) and runs the 8 transformer layers plus the vocab
head entirely out of SBUF/PSUM.

Perf notes (vs v0): LN statistics are computed with an all-ones (1/D-scaled)
[128,128] stationary matmul so mean/E[x^2] arrive already broadcast across
partitions (no fp32 broadcast matmuls, no [1,N] row pipeline); inverse std
comes from ScalarE Ln+Exp; softmax denominators use the fast approximate
reciprocal; the causal mask is folded into the QK^T accumulation as one
extra [128,128] matmul (upper_strict^T @ (-400*I)); attention heads are
software-pipelined so QK^T of head h+1 covers the exp/denominator latency
of head h; logits are emitted bf16 and upcast on host.
"""
import sys
sys.path.insert(0, '/opt/trn_rl_repo')
import numpy as np
import ml_dtypes

import concourse.bass as bass
import concourse.tile as tile
from concourse import bacc, mybir
from concourse.bass_utils import run_bass_kernel_spmd

B, T, D, H, L, V, MAXT = 8, 1024, 1024, 16, 8, 8192, 4096
HD = D // H          # 64
P = 128
DS = D // P          # 8 d-subtiles
TS = T // P          # 8 t-subtiles
D2S = (2 * D) // P   # 16 mlp subtiles
VS = V // 512        # 16 vocab chunks
NCH = 512
EPS = 1e-5
SCALE = 1.0 / np.sqrt(HD)
NEGMASK = -400.0

F32 = mybir.dt.float32
BF16 = mybir.dt.bfloat16
AF = mybir.ActivationFunctionType
ALU = mybir.AluOpType

# smalls[:, col] layout, per layer base = l*48 (bias columns; only used when
# the corresponding bias is nonzero)
SM_BQ, SM_BK, SM_BO, SM_B2, SM_B1 = 0, 8, 16, 24, 32
SM_PER_LAYER = 48
SM_COLS = L * 48

TRACE = False
LAST_RESULTS = None


def _build(flags, repeat=1, debug=False):
    """flags: (zq, zk, zv, zo, zb1, zb2, zhb) — True = that bias is all-zero."""
    import contextlib
    zq, zk, zv, zo, zb1, zb2, zhb = flags
    nc = bacc.Bacc("TRN2", target_bir_lowering=False)
    dbg = {}
    if debug:
        for name, shp, dt in (("dbg_h1", [P, DS, T], BF16), ("dbg_qk", [P, 2 * DS, T], BF16),
                              ("dbg_vg", [P, TS, H, HD + 1], BF16), ("dbg_y", [P, DS, T], BF16),
                              ("dbg_xa", [P, DS, T], F32), ("dbg_g", [P, 2 * DS, T], BF16),
                              ("dbg_x1", [P, DS, T], F32), ("dbg_hf", [P, DS, T], BF16)):
            dbg[name] = nc.dram_tensor(name, shp, dt, kind="ExternalOutput")

    x0T_d = nc.dram_tensor("x0T", [D, T], F32, kind="ExternalInput")
    WqT_d = nc.dram_tensor("WqT", [L, D, D], BF16, kind="ExternalInput")
    WkT_d = nc.dram_tensor("WkT", [L, D, D], BF16, kind="ExternalInput")
    WvT_d = nc.dram_tensor("WvT", [L, D, D], BF16, kind="ExternalInput")
    WoT_d = nc.dram_tensor("WoT", [L, D, D], BF16, kind="ExternalInput")
    W1T_d = nc.dram_tensor("W1T", [L, D, 2 * D], BF16, kind="ExternalInput")
    W2T_d = nc.dram_tensor("W2T", [L, 2 * D, D], BF16, kind="ExternalInput")
    hT_w_d = nc.dram_tensor("headT", [D, V], BF16, kind="ExternalInput")
    sm_d = nc.dram_tensor("smalls", [P, SM_COLS], F32, kind="ExternalInput")
    bv_d = nc.dram_tensor("bvB", [1, L * D], BF16, kind="ExternalInput")
    um_d = nc.dram_tensor("umask", [P, P], BF16, kind="ExternalInput")
    ni_d = nc.dram_tensor("negI", [P, P], BF16, kind="ExternalInput")
    hb_d = nc.dram_tensor("headB", [1, V], BF16, kind="ExternalInput")
    out_d = nc.dram_tensor("logits", [T, V], BF16, kind="ExternalOutput")

    out_r = out_d[:, :].rearrange("(t pi) v -> pi t v", pi=P)

    with tile.TileContext(nc) as tc:
        with (
            tc.tile_pool(name="pc", bufs=1) as pc,
            tc.tile_pool(name="pw", bufs=4) as pw,
            tc.tile_pool(name="pbv", bufs=2) as pbv,
            tc.tile_pool(name="ppt", bufs=2) as ppt,
            tc.tile_pool(name="px", bufs=3) as px,
            tc.tile_pool(name="pn", bufs=4) as pn,
            tc.tile_pool(name="pv", bufs=3) as pv,
            tc.tile_pool(name="pr", bufs=2) as pr,
            tc.tile_pool(name="pps", bufs=4, space="PSUM") as pps,
            tc.tile_pool(name="ppo", bufs=2, space="PSUM") as ppo,
            tc.tile_pool(name="ppb", bufs=2, space="PSUM") as ppb,
        ):
            xT = pc.tile([P, DS, T], F32)
            hT = pc.tile([P, DS, T], BF16)    # xb -> x_hat -> yT, per phase
            qkT = pc.tile([P, 2 * DS, T], BF16)  # q 0:8, k 8:16; mlp g 0:16
            Vg = pc.tile([P, TS, H, HD + 1], BF16)
            yT = hT
            sm = pc.tile([P, SM_COLS], F32)
            um = pc.tile([P, P], BF16)        # strict upper triangular ones
            negI = pc.tile([P, P], BF16)      # -400 * I
            onesD = pc.tile([P, P], BF16)     # all 1/D
            ones_rb = pc.tile([1, HD], BF16)
            ones_r = pc.tile([1, P], BF16)
            hbrow = pc.tile([1, V], BF16)
            eps_t = pc.tile([P, 1], F32)

            nc.vector.memset(eps_t[:], EPS)
            nc.vector.memset(onesD[:], 1.0 / D)
            nc.vector.memset(ones_rb[:], 1.0)
            nc.vector.memset(ones_r[:], 1.0)
            nc.vector.memset(Vg[:, :, :, HD:HD + 1], 1.0)
            nc.sync.dma_start(sm[:], sm_d[:, :])
            nc.sync.dma_start(um[:], um_d[:, :])
            nc.sync.dma_start(negI[:], ni_d[:, :])
            if not zhb:
                nc.sync.dma_start(hbrow[:], hb_d[:, :])

            def cast_x(dst, koff, k, c):
                # bf16 copy of residual stream subtile for LN stats
                tch = bass.ts(c, NCH)
                nc.vector.tensor_copy(dst[:, koff + k, tch], xT[:, k, tch])

            def ln_stats(xb, koff, c):
                """-> (S_psum, Q_psum): mean and E[x^2], broadcast [P, NCH]."""
                tch = bass.ts(c, NCH)
                S = pps.tile([P, NCH], F32, tag="a")
                Q = pps.tile([P, NCH], F32, tag="a")
                sqs = []
                for k in range(DS):
                    sq = px.tile([P, NCH], BF16, tag="sq")
                    nc.scalar.activation(sq[:], xT[:, k, tch], AF.Square)
                    sqs.append(sq)
                for k in range(DS):
                    nc.tensor.matmul(S[:], onesD[:], xb[:, koff + k, tch],
                                     start=(k == 0), stop=(k == DS - 1))
                for k in range(DS):
                    nc.tensor.matmul(Q[:], onesD[:], sqs[k][:],
                                     start=(k == 0), stop=(k == DS - 1))
                return S, Q

            def ln_var(S, Q):
                m2 = pv.tile([P, NCH], F32, tag="v")
                nc.scalar.activation(m2[:], S[:], AF.Square)
                var = pv.tile([P, NCH], F32, tag="v")
                nc.vector.tensor_sub(var[:], Q[:], m2[:])
                return var

            def ln_srow(S, var):
                lnv = pv.tile([P, NCH], F32, tag="v")
                nc.scalar.activation(lnv[:], var[:], AF.Ln, bias=eps_t[:])
                srow = pn.tile([P, NCH], F32, tag="n")
                nc.scalar.activation(srow[:], lnv[:], AF.Exp, scale=-0.5)
                nmb = pn.tile([P, NCH], F32, tag="n")
                nc.vector.scalar_tensor_tensor(nmb[:], S[:], -1.0, srow[:],
                                               op0=ALU.mult, op1=ALU.mult)
                return srow, nmb

            def ln_norm(srow, nmb, dst, c):
                """dst[:, k, chunk c] = (x - mean) / std  (gains folded into
                the consuming weights on host). Split DVE/GpSimd so the first
                consumer chain is not serialized behind one engine."""
                tch = bass.ts(c, NCH)
                for k in range(DS):
                    eng = nc.gpsimd if k % 3 == 1 else nc.vector
                    u = px.tile([P, NCH], F32, tag="u")
                    eng.tensor_mul(u[:], xT[:, k, tch], srow[:])
                    eng.tensor_add(dst[:, k, tch], u[:], nmb[:])

            def layernorm(xb, koff, dst):
                # squares (one ScalarE table set) first, then Ln/Exp pairs
                S0, Q0 = ln_stats(xb, koff, 0)
                S1, Q1 = ln_stats(xb, koff, 1)
                var0 = ln_var(S0, Q0)
                var1 = ln_var(S1, Q1)
                srow0, nmb0 = ln_srow(S0, var0)
                srow1, nmb1 = ln_srow(S1, var1)
                ln_norm(srow0, nmb0, dst, 0)
                ln_norm(srow1, nmb1, dst, 1)

            loop_cm = tc.For_i(0, repeat, 1) if repeat > 1 else contextlib.nullcontext()
            with loop_cm:
                nc.sync.dma_start(xT[:], x0T_d[:, :].rearrange("(po pi) t -> pi po t", pi=P))
                for c in range(2):
                    for k in range(DS):
                        cast_x(hT, 0, k, c)

                def dump(name, tl):
                    if debug and name in dbg:
                        d = dbg[name]
                        sl = (slice(None),) * len(d.shape)
                        nc.sync.dma_start(d[sl], tl[:])

                for l in range(L):
                    base = l * SM_PER_LAYER
                    layernorm(hT, 0, hT)
                    if l == 0:
                        dump("dbg_h1", hT)

                    # ---- q^T / k^T projections: out[o, t] ----
                    for which, W_d, boff, qoff, zb in (
                            (0, WqT_d, SM_BQ, 0, zq), (1, WkT_d, SM_BK, DS, zk)):
                        for half in range(2):
                            wsl = pw.tile([P, DS, NCH], BF16, tag="w")
                            nc.sync.dma_start(
                                wsl[:],
                                W_d[l].rearrange("(po pi) o -> pi po o", pi=P)[:, :, bass.ts(half, NCH)])
                            for c in range(2):
                                for m in range(4):
                                    mo = half * 4 + m
                                    ps = pps.tile([P, NCH], F32, tag="a")
                                    for k in range(DS):
                                        nc.tensor.matmul(ps[:], wsl[:, k, bass.ts(m, P)],
                                                         hT[:, k, bass.ts(c, NCH)],
                                                         start=(k == 0), stop=(k == DS - 1))
                                    if zb:
                                        nc.vector.tensor_copy(
                                            qkT[:, qoff + mo, bass.ts(c, NCH)], ps[:])
                                    else:
                                        nc.vector.tensor_scalar_add(
                                            qkT[:, qoff + mo, bass.ts(c, NCH)], ps[:],
                                            sm[:, base + boff + mo:base + boff + mo + 1])

                    # ---- V projection: out[t, o] (natural) into Vg ----
                    for half in range(2):
                        wsl = pw.tile([P, DS, NCH], BF16, tag="w")
                        nc.sync.dma_start(
                            wsl[:],
                            WvT_d[l].rearrange("(po pi) o -> pi po o", pi=P)[:, :, bass.ts(half, NCH)])
                        if not zv:
                            bvs = pbv.tile([1, NCH], BF16, tag="bvs")
                            nc.sync.dma_start(bvs[:], bv_d[:, l * D + half * NCH:l * D + (half + 1) * NCH])
                        for t_ in range(TS):
                            ps_t = pps.tile([P, NCH], F32, tag="a")
                            for k in range(DS):
                                nc.tensor.matmul(ps_t[:], hT[:, k, bass.ts(t_, P)],
                                                 wsl[:, k, :],
                                                 start=(k == 0), stop=(zv and k == DS - 1))
                            if not zv:
                                nc.tensor.matmul(ps_t[:], ones_r[:], bvs[:],
                                                 start=False, stop=True)
                            nc.vector.tensor_copy(
                                Vg[:, t_, 8 * half:8 * half + 8, 0:HD],
                                ps_t[:].rearrange("p (h d) -> p h d", d=HD))

                    # ---- attention: heads pipelined within each chunk ----
                    def stage_a(h, c):
                        pbase = (h % 2) * HD
                        sub = h // 2
                        PT = ppt.tile([P, TS, NCH], BF16, tag="pt")
                        ntk = 4 * c + 4
                        for tk in range(ntk):
                            ls = max(0, tk * P - c * NCH)
                            w_ = NCH - ls
                            masked = tk >= 4 * c
                            sT = pps.tile([P, NCH], F32, tag="a")
                            nc.tensor.matmul(
                                sT[:, :w_],
                                qkT[pbase:pbase + HD, DS + sub, bass.ts(tk, P)],
                                qkT[pbase:pbase + HD, sub, c * NCH + ls:(c + 1) * NCH],
                                start=True, stop=not masked)
                            if masked:
                                nc.tensor.matmul(sT[:, 0:P], um[:], negI[:],
                                                 start=False, stop=True)
                            nc.scalar.activation(PT[:, tk, ls:], sT[:, :w_], AF.Exp,
                                                 scale=float(SCALE))
                        return PT

                    def stage_b1(h, c, PT):
                        ntk = 4 * c + 4
                        po = ppo.tile([HD + 1, NCH], F32, tag="o")
                        for tk in range(ntk):
                            ls = max(0, tk * P - c * NCH)
                            nc.tensor.matmul(po[:, ls:], Vg[:, tk, h, :],
                                             PT[:, tk, ls:],
                                             start=(tk == 0), stop=(tk == ntk - 1))
                        dns = pr.tile([1, NCH], F32, tag="rs")
                        nc.vector.tensor_copy(dns[:], po[HD:HD + 1, :])
                        dn = pr.tile([1, NCH], F32, tag="r")
                        nc.vector.reciprocal_approx_fast(dn[:], dns[:])
                        dnb = pr.tile([1, NCH], BF16, tag="rb")
                        nc.vector.tensor_copy(dnb[:], dn[:])
                        return (h, po, dnb)

                    def stage_b2(h, po, dnb, c):
                        pbase = (h % 2) * HD
                        sub = h // 2
                        bc = ppb.tile([HD, NCH], F32, tag="b")
                        nc.tensor.matmul(bc[:], ones_rb[:], dnb[:],
                                         start=True, stop=True)
                        yu = px.tile([HD, NCH], BF16, tag="yu")
                        nc.vector.tensor_copy(yu[:], po[:HD, :])
                        nc.vector.tensor_mul(yT[pbase:pbase + HD, sub, bass.ts(c, NCH)],
                                             yu[:], bc[:])

                    if l == 0:
                        dump("dbg_qk", qkT)
                        dump("dbg_vg", Vg)
                    for c in range(2):
                        prev_a = None
                        prev_b = None
                        for h in range(H):
                            PT = stage_a(h, c)
                            if prev_b is not None:
                                stage_b2(*prev_b, c)
                                prev_b = None
                            if prev_a is not None:
                                prev_b = stage_b1(prev_a[0], c, prev_a[1])
                            prev_a = (h, PT)
                        if prev_b is not None:
                            stage_b2(*prev_b, c)
                        prev_b = stage_b1(prev_a[0], c, prev_a[1])
                        stage_b2(*prev_b, c)
                    if l == 0:
                        dump("dbg_y", hT)

                    # ---- attention out projection + residual ----
                    for half in range(2):
                        wsl = pw.tile([P, DS, NCH], BF16, tag="w")
                        nc.sync.dma_start(
                            wsl[:],
                            WoT_d[l].rearrange("(po pi) o -> pi po o", pi=P)[:, :, bass.ts(half, NCH)])
                        for c in range(2):
                            tch = bass.ts(c, NCH)
                            for m in range(4):
                                mo = half * 4 + m
                                ps = pps.tile([P, NCH], F32, tag="a")
                                for k in range(DS):
                                    nc.tensor.matmul(ps[:], wsl[:, k, bass.ts(m, P)],
                                                     yT[:, k, tch],
                                                     start=(k == 0), stop=(k == DS - 1))
                                if zo:
                                    nc.vector.tensor_add(xT[:, mo, tch], ps[:], xT[:, mo, tch])
                                else:
                                    nc.vector.scalar_tensor_tensor(
                                        xT[:, mo, tch], ps[:],
                                        sm[:, base + SM_BO + mo:base + SM_BO + mo + 1],
                                        xT[:, mo, tch], op0=ALU.add, op1=ALU.add)
                                # eager bf16 copy for LN2 stats (k rows of qkT are dead)
                                cast_x(qkT, DS, mo, c)
                    if l == 0:
                        dump("dbg_xa", xT)

                    # ---- MLP ----
                    layernorm(qkT, DS, hT)
                    for quarter in range(4):
                        wsl = pw.tile([P, DS, NCH], BF16, tag="w")
                        nc.sync.dma_start(
                            wsl[:],
                            W1T_d[l].rearrange("(po pi) o -> pi po o", pi=P)[:, :, bass.ts(quarter, NCH)])
                        for c in range(2):
                            for m in range(4):
                                mo = quarter * 4 + m
                                ps = pps.tile([P, NCH], F32, tag="a")
                                for k in range(DS):
                                    nc.tensor.matmul(ps[:], wsl[:, k, bass.ts(m, P)],
                                                     hT[:, k, bass.ts(c, NCH)],
                                                     start=(k == 0), stop=(k == DS - 1))
                                if zb1:
                                    nc.scalar.activation(
                                        qkT[:, mo, bass.ts(c, NCH)], ps[:], AF.Gelu)
                                else:
                                    nc.scalar.activation(
                                        qkT[:, mo, bass.ts(c, NCH)], ps[:], AF.Gelu,
                                        bias=sm[:, base + SM_B1 + mo:base + SM_B1 + mo + 1])

                    if l == 0:
                        dump("dbg_g", qkT)
                    for half in range(2):
                        w2r = W2T_d[l].rearrange("(po pi) o -> pi po o", pi=P)
                        w2a = pw.tile([P, DS, NCH], BF16, tag="w")
                        nc.sync.dma_start(w2a[:], w2r[:, 0:DS, bass.ts(half, NCH)])
                        w2b = pw.tile([P, DS, NCH], BF16, tag="w")
                        nc.sync.dma_start(w2b[:], w2r[:, DS:D2S, bass.ts(half, NCH)])
                        for c in range(2):
                            tch = bass.ts(c, NCH)
                            for m in range(4):
                                mo = half * 4 + m
                                ps = pps.tile([P, NCH], F32, tag="a")
                                for k in range(D2S):
                                    wt = w2a if k < DS else w2b
                                    nc.tensor.matmul(ps[:], wt[:, k % DS, bass.ts(m, P)],
                                                     qkT[:, k, tch],
                                                     start=(k == 0), stop=(k == D2S - 1))
                                if zb2:
                                    nc.vector.tensor_add(xT[:, mo, tch], ps[:], xT[:, mo, tch])
                                else:
                                    nc.vector.scalar_tensor_tensor(
                                        xT[:, mo, tch], ps[:],
                                        sm[:, base + SM_B2 + mo:base + SM_B2 + mo + 1],
                                        xT[:, mo, tch], op0=ALU.add, op1=ALU.add)
                                # eager bf16 copy for next LN1 / final LN (hT: W1 done)
                                cast_x(hT, 0, mo, c)
                    if l == 0:
                        dump("dbg_x1", xT)

                # ---- final LN + vocab head ----
                layernorm(hT, 0, hT)
                dump("dbg_hf", hT)
                hw_r = hT_w_d[:, :].rearrange("(po pi) v -> pi po v", pi=P)
                for vp in range(VS // 2):
                    ws0 = pw.tile([P, DS, NCH], BF16, tag="w")
                    nc.sync.dma_start(ws0[:], hw_r[:, :, bass.ts(2 * vp, NCH)])
                    ws1 = pw.tile([P, DS, NCH], BF16, tag="w")
                    nc.sync.dma_start(ws1[:], hw_r[:, :, bass.ts(2 * vp + 1, NCH)])
                    for t_ in range(TS):
                        ps0 = pps.tile([P, NCH], F32, tag="a")
                        ps1 = pps.tile([P, NCH], F32, tag="a")
                        for k in range(DS):
                            nc.tensor.matmul(ps0[:], hT[:, k, bass.ts(t_, P)],
                                             ws0[:, k, :],
                                             start=(k == 0), stop=(zhb and k == DS - 1))
                            nc.tensor.matmul(ps1[:], hT[:, k, bass.ts(t_, P)],
                                             ws1[:, k, :],
                                             start=(k == 0), stop=(zhb and k == DS - 1))
                        if not zhb:
                            nc.tensor.matmul(ps0[:], ones_r[:],
                                             hbrow[:, bass.ts(2 * vp, NCH)],
                                             start=False, stop=True)
                            nc.tensor.matmul(ps1[:], ones_r[:],
                                             hbrow[:, bass.ts(2 * vp + 1, NCH)],
                                             start=False, stop=True)
                        for j, psx in ((0, ps0), (1, ps1)):
                            ot = px.tile([P, NCH], BF16, tag="ot")
                            nc.scalar.copy(ot[:], psx[:])
                            nc.sync.dma_start(out_r[:, t_, bass.ts(2 * vp + j, NCH)], ot[:])

    nc.compile()
    return nc


_NC = {}


def _get_nc(flags=(True,) * 7, repeat=1):
    key = (flags, repeat)
    if key not in _NC:
        _NC[key] = _build(flags, repeat)
    return _NC[key]


def _pack_cols(vec, ncols):
    """[ncols*128] -> [128, ncols] with column j = vec[j*128:(j+1)*128]."""
    return np.ascontiguousarray(vec.reshape(ncols, P).T)


LAST_IN_MAPS = None
LAST_FLAGS = None


def kernel(idx, timesteps, tok_emb_w, pos_emb, global_pos_emb,
           ln1_w, ln1_b, Wq, bq, Wk, bk, Wv, bv, Wo, bo,
           ln2_w, ln2_b, W1, b1, W2, b2, lnf_w, lnf_b, head_w):
    global LAST_RESULTS, LAST_IN_MAPS, LAST_FLAGS
    f = lambda a: np.asarray(a, dtype=np.float32)
    idx = np.asarray(idx, dtype=np.int64)
    tsteps = np.asarray(timesteps, dtype=np.int64)
    tok_emb_w, pos_emb, global_pos_emb = f(tok_emb_w), f(pos_emb), f(global_pos_emb)
    ln1_w, ln1_b = f(ln1_w), f(ln1_b)
    ln2_w, ln2_b = f(ln2_w), f(ln2_b)
    lnf_w, lnf_b = f(lnf_w), f(lnf_b)
    Wq, Wk, Wv, Wo, W1, W2, head_w = map(f, (Wq, Wk, Wv, Wo, W1, W2, head_w))
    bq, bk, bv, bo, b1, b2 = map(f, (bq, bk, bv, bo, b1, b2))

    # embedding on host (tiny compute, avoids on-device gather)
    x0 = tok_emb_w[idx] + global_pos_emb[0][tsteps[:, 0]][:, None, :] + pos_emb[:, :T]
    x0 = np.ascontiguousarray(x0.astype(np.float32))

    # fold LN gains/biases into the consuming weight matrices
    Wq_f = Wq * ln1_w[:, None, :]
    Wk_f = Wk * ln1_w[:, None, :]
    Wv_f = Wv * ln1_w[:, None, :]
    W1_f = W1 * ln2_w[:, None, :]
    hw_f = head_w * lnf_w[None, :]
    bq_f = bq + np.einsum('lod,ld->lo', Wq, ln1_b)
    bk_f = bk + np.einsum('lod,ld->lo', Wk, ln1_b)
    bv_f = bv + np.einsum('lod,ld->lo', Wv, ln1_b)
    b1_f = b1 + np.einsum('lod,ld->lo', W1, ln2_b)
    hb_f = head_w @ lnf_b

    flags = (not bq_f.any(), not bk_f.any(), not bv_f.any(), not bo.any(),
             not b1_f.any(), not b2.any(), not hb_f.any())

    bf = lambda a: np.ascontiguousarray(np.asarray(a, np.float32)).astype(ml_dtypes.bfloat16)
    shared = {
        "WqT": bf(Wq_f.transpose(0, 2, 1)),
        "WkT": bf(Wk_f.transpose(0, 2, 1)),
        "WvT": bf(Wv_f.transpose(0, 2, 1)),
        "WoT": bf(Wo.transpose(0, 2, 1)),
        "W1T": bf(W1_f.transpose(0, 2, 1)),
        "W2T": bf(W2.transpose(0, 2, 1)),
        "headT": bf(hw_f.T),
        "bvB": bf(bv_f.reshape(1, L * D)),
        "headB": bf(hb_f.reshape(1, V)),
    }
    smalls = np.zeros((P, SM_COLS), np.float32)
    for l in range(L):
        b = l * SM_PER_LAYER
        smalls[:, b + SM_BQ:b + SM_BQ + 8] = _pack_cols(bq_f[l], DS)
        smalls[:, b + SM_BK:b + SM_BK + 8] = _pack_cols(bk_f[l], DS)
        smalls[:, b + SM_BO:b + SM_BO + 8] = _pack_cols(bo[l], DS)
        smalls[:, b + SM_B2:b + SM_B2 + 8] = _pack_cols(b2[l], DS)
        smalls[:, b + SM_B1:b + SM_B1 + 16] = _pack_cols(b1_f[l], D2S)
    shared["smalls"] = smalls

    shared["umask"] = (np.arange(P)[:, None] < np.arange(P)[None, :]).astype(ml_dtypes.bfloat16)
    shared["negI"] = (NEGMASK * np.eye(P)).astype(ml_dtypes.bfloat16)

    in_maps = []
    for b_ in range(B):
        m = dict(shared)
        m["x0T"] = np.ascontiguousarray(x0[b_].T)
        in_maps.append(m)

    LAST_IN_MAPS = in_maps
    LAST_FLAGS = flags
    nc = _get_nc(flags)
    res = run_bass_kernel_spmd(nc, in_maps, core_ids=list(range(B)), trace=TRACE)
    LAST_RESULTS = res
    out = np.stack([np.asarray(res.results[c]["logits"]).astype(np.float32)
                    for c in range(B)])
    return out


# ---------------------------------------------------------------------------
# Timing helpers (test-only): replicate run_bass_via_pjrt's sharded jit with
# device-resident inputs so repeated calls measure (dispatch + NEFF exec).
# ---------------------------------------------------------------------------
def _sharded_exec(nc, in_maps):
    import jax
    from jax.experimental.shard_map import shard_map
    from jax.sharding import Mesh, PartitionSpec
    from concourse import bass2jax

    bass2jax.install_neuronx_cc_hook()
    n_cores = len(in_maps)
    partition_name = nc.partition_id_tensor.name if nc.partition_id_tensor else None
    in_names, out_names, out_avals, zero_outs = [], [], [], []
    for alloc in nc.m.functions[0].allocations:
        if not isinstance(alloc, mybir.MemoryLocationSet):
            continue
        name = alloc.memorylocations[0].name
        if alloc.kind == "ExternalInput":
            if name != partition_name:
                in_names.append(name)
        elif alloc.kind == "ExternalOutput":
            shape = tuple(alloc.tensor_shape)
            dtype = mybir.dt.np(alloc.dtype)
            out_names.append(name)
            out_avals.append(jax.core.ShapedArray(shape, dtype))
            zero_outs.append(np.zeros(shape, dtype))
    n_params = len(in_names)
    n_outs = len(out_avals)
    all_in_names = list(in_names) + list(out_names)
    if partition_name is not None:
        all_in_names.append(partition_name)
    donate = tuple(range(n_params, n_params + n_outs))

    def _body(*args):
        operands = list(args)
        if partition_name is not None:
            operands.append(bass2jax.partition_id_tensor())
        outs = bass2jax._bass_exec_p.bind(
            *operands,
            out_avals=tuple(out_avals),
            in_names=tuple(all_in_names),
            out_names=tuple(out_names),
            lowering_input_output_aliases=(),
            sim_require_finite=True,
            sim_require_nnan=True,
            nc=nc,
        )
        return tuple(outs)

    devices = jax.devices()[:n_cores]
    mesh = Mesh(np.asarray(devices), ("core",))
    sharded = jax.jit(
        shard_map(_body, mesh=mesh,
                  in_specs=(PartitionSpec("core"),) * (n_params + n_outs),
                  out_specs=(PartitionSpec("core"),) * n_outs,
                  check_rep=False),
        donate_argnums=donate, keep_unused=True)

    concat_in = [np.concatenate([np.asarray(m[name]) for m in in_maps], axis=0)
                 for name in in_names]
    concat_zeros = [np.zeros((n_cores * z.shape[0], *z.shape[1:]), z.dtype)
                    for z in zero_outs]
    from jax.sharding import NamedSharding
    sh = NamedSharding(mesh, PartitionSpec("core"))
    dev_in = [jax.device_put(a, sh) for a in concat_in]
    return sharded, dev_in, concat_zeros, sh


def _time_exec(nc, in_maps, iters):
    import time as _time
    import jax
    sharded, dev_in, concat_zeros, sh = _sharded_exec(nc, in_maps)
    times = []
    for _ in range(iters):
        zs = [jax.device_put(z, sh) for z in concat_zeros]
        jax.block_until_ready(zs)
        jax.block_until_ready(dev_in)
        t0 = _time.perf_counter()
        out = sharded(*dev_in, *zs)
        jax.block_until_ready(out)
        times.append(_time.perf_counter() - t0)
    return times


def timed_run(iters=5):
    assert LAST_IN_MAPS is not None, "call kernel() first"
    return _time_exec(_get_nc(LAST_FLAGS), LAST_IN_MAPS, iters)


def timed_slope(ns=(1, 4, 12), zsets=12):
    """Async-dispatch n calls back-to-back; slope of total-time vs n ~ exec."""
    import time as _time
    import jax
    assert LAST_IN_MAPS is not None
    sharded, dev_in, concat_zeros, sh = _sharded_exec(_get_nc(LAST_FLAGS), LAST_IN_MAPS)
    all_zs = [[jax.device_put(z, sh) for z in concat_zeros] for _ in range(1)]
    jax.block_until_ready(all_zs)
    jax.block_until_ready(dev_in)
    out = sharded(*dev_in, *all_zs[0])
    jax.block_until_ready(out)
    res = {}
    for n in ns:
        zs_fresh = [[jax.device_put(z, sh) for z in concat_zeros] for _ in range(n)]
        jax.block_until_ready(zs_fresh)
        t0 = _time.perf_counter()
        outs = [sharded(*dev_in, *zs_fresh[i]) for i in range(n)]
        jax.block_until_ready(outs)
        res[n] = _time.perf_counter() - t0
    return res


def timed_repeat(r=5, iters=6):
    """exec_ns ~= (min_time(R=r NEFF) - min_time(R=1 NEFF)) / (r-1)."""
    assert LAST_IN_MAPS is not None
    t1 = min(_time_exec(_get_nc(LAST_FLAGS, 1), LAST_IN_MAPS, iters))
    tr = min(_time_exec(_get_nc(LAST_FLAGS, r), LAST_IN_MAPS, iters))
    return (tr - t1) / (r - 1), t1, tr
